# revision 21
# baseline (speedup 1.0000x reference)
"""Trainium2 Bass kernel for per-sample masked conv2d (dynamic weight attention conv).

out[b] = conv2d(x[b], weight * m[b], stride=1, pad=1) + bias

Strategy: pure data parallel over batch (32 samples -> 8 cores x 4 samples).
Per sample the conv runs as 18 accumulation stages (2 input-channel chunks x 9
taps) of matmuls over 7 row-group PSUM accumulators, so each stationary weight
load serves 7 consecutive matmuls.  The datapath is bf16: masked weights are
built by a DVE multiply (f32 m x f32 w -> bf16), transposed 128x128-tile-wise
on the TensorEngine into [i, o] stationary layout, packed 8-per-PSUM-bank and
drained by the Activation engine.  Transposes for sample s+1 are interleaved
between sample s's matmul stages so the PE never idles.
"""

import sys
from contextlib import ExitStack

for _p in ("/opt/trn_rl_repo",):
    if _p not in sys.path:
        sys.path.append(_p)

import numpy as np

import concourse.bass as bass
import concourse.mybir as mybir
import concourse.tile as tile
from concourse import bacc, bass_utils

# NOTE: walrus --enable-ldw-opt rejects the standalone InstLdweights that
# bass emits for non-f32 stationary dtypes, so it stays at its default
# (false) for this bf16 kernel.

# Problem constants (hardcoded per contract)
B, FIN, FOUT, KK, H, W = 32, 256, 256, 3, 56, 56
N_CORES = 8
BPC = B // N_CORES          # samples per core = 4
P = 128                     # partition width
NI = FIN // P               # input-channel chunks = 2
NO = FOUT // P              # output-channel chunks = 2
HP, WP = H + 2, W + 2       # padded spatial = 58x58
RG = 8                      # output rows per row-group
NRG = H // RG               # row groups = 7
NT = RG * W                 # matmul moving free size = 448
KSQ = KK * KK               # 9
CFREE = FIN * KSQ           # 2304
HALF = CFREE // NI          # 1152
NST = NO * NI * KSQ         # 36 weight stages per sample
RH = H // 2                 # 28
F32 = mybir.dt.float32
BF16 = mybir.dt.bfloat16



def build_program():
    nc = bacc.Bacc("TRN2", target_bir_lowering=False, debug=False,
                   num_devices=N_CORES)

    x_d = nc.dram_tensor("x", [BPC, FIN, H, W], F32, kind="ExternalInput").ap()
    m_d = nc.dram_tensor("m", [BPC, FOUT, FIN, KK, KK], F32,
                         kind="ExternalInput").ap()
    w_d = nc.dram_tensor("weight", [FOUT, FIN, KK, KK], F32,
                         kind="ExternalInput").ap()
    b_d = nc.dram_tensor("bias", [FOUT], F32, kind="ExternalInput").ap()
    o_d = nc.dram_tensor("out", [BPC, FOUT, H, W], F32,
                         kind="ExternalOutput").ap()

    x_nat = x_d.rearrange("s (c p) h w -> s c p h w", p=P)
    m_nat = m_d.rearrange("s (c p) i kh kw -> s c p (i kh kw)", p=P)
    w_nat = w_d.rearrange("(c p) i kh kw -> c p (i kh kw)", p=P)
    o_nat = o_d.rearrange("s (c p) h w -> s c p (h w)", p=P)

    with tile.TileContext(nc) as tc, ExitStack() as ctx:
        consts = ctx.enter_context(tc.tile_pool(name="consts", bufs=1))
        m_pool = ctx.enter_context(tc.tile_pool(name="m_pool", bufs=2))
        mw_pool = ctx.enter_context(tc.tile_pool(name="mw_pool", bufs=2))
        xs_pool = ctx.enter_context(tc.tile_pool(name="xs_pool", bufs=2))
        xp_pool = ctx.enter_context(tc.tile_pool(name="xp_pool", bufs=2 * NI))
        wt_pool = ctx.enter_context(tc.tile_pool(name="wt_pool", bufs=2))
        osb_pool = ctx.enter_context(tc.tile_pool(name="osb_pool", bufs=2))
        acc_psum = ctx.enter_context(tc.tile_pool(name="acc_psum", bufs=8,
                                                  space="PSUM"))

        # weight tiles, natural layout f32 [oc][128, (i kh kw)]; DMAs are
        # emitted in the prologue interleaved with sample 0's m/x loads on
        # the sync ring, in the exact order the compute consumes them (the
        # DMA engines fair-share bandwidth across outstanding transfers, so
        # issue order controls completion order of the critical first chunks)
        w_tiles = [consts.tile([P, CFREE], F32, name=f"w_{oc}")
                   for oc in range(NO)]
        bias_t = consts.tile([P, NO], F32, name="bias_t")

        def emit_w_load(oc, h, eng=None):
            sl = slice(h * HALF, (h + 1) * HALF)
            (eng or nc.sync).dma_start(out=w_tiles[oc][:, sl],
                                       in_=w_nat[oc][:, sl])

        # per-sample state
        mw_tiles = {}   # s -> [oc] bf16 [P, CFREE]
        mwT = {}        # s -> bf16 [P, NST*P] stationary store
        xp_tiles = {}   # s -> [ic] bf16 [P, HP, WP]

        def stage_params(t):
            oc, r = divmod(t, NI * KSQ)
            ic, k = divmod(r, KSQ)
            kh, kw = divmod(k, KK)
            return oc, ic, kh, kw

        def emit_m_loads(s, eng=None):
            eng = eng or nc.sync
            mts = []
            for oc in range(NO):
                mt = m_pool.tile([P, CFREE], F32, name=f"m_{s}_{oc}", tag="m")
                for h in range(NI):
                    sl = slice(h * HALF, (h + 1) * HALF)
                    eng.dma_start(out=mt[:, sl], in_=m_nat[s, oc][:, sl])
                mts.append(mt)
            return mts

        def emit_x_loads(s, eng=None):
            eng = eng or nc.scalar
            xss = []
            for ic in range(NI):
                xs = xs_pool.tile([P, H * W], F32, name=f"xs_{s}_{ic}",
                                  tag="xs")
                eng.dma_start(out=xs[:, :RH * W], in_=x_nat[s, ic][:, :RH, :])
                eng.dma_start(out=xs[:, RH * W:], in_=x_nat[s, ic][:, RH:, :])
                xss.append(xs)
            return xss

        def emit_mul_alloc(s, oc):
            mw = mw_pool.tile([P, CFREE], BF16, name=f"mw_{s}_{oc}", tag="mw")
            mw_tiles.setdefault(s, []).append(mw)
            return mw

        def emit_mul_half(s, mts, oc, h):
            # masked weight in (k-major, i-minor) free layout: the xbar DMA
            # transpose then turns each 128-column block (= one tap k) into
            # the [i, o] stationary tile the conv matmul needs
            mw = mw_tiles[s][oc]
            sl = slice(h * HALF, (h + 1) * HALF)
            out = mw[:, sl].rearrange("p (k i) -> p i k", i=P)
            m_in = mts[oc][:, sl].rearrange("p (i k) -> p i k", k=KSQ)
            w_in = w_tiles[oc][:, sl].rearrange("p (i k) -> p i k", k=KSQ)
            nc.vector.tensor_mul(out, m_in, w_in)

        def emit_mul(s, mts, oc):
            emit_mul_alloc(s, oc)
            for h in range(NI):
                emit_mul_half(s, mts, oc, h)

        def emit_xp_borders(s, ic, xp):
            # zero halo via gpsimd (idle engine); interior overwritten later
            nc.gpsimd.memset(xp[:, 0, :], 0.0)
            nc.gpsimd.memset(xp[:, HP - 1, :], 0.0)
            nc.gpsimd.memset(xp[:, 1:HP - 1, 0:1], 0.0)
            nc.gpsimd.memset(xp[:, 1:HP - 1, WP - 1:WP], 0.0)

        def emit_xp_alloc(s, ic):
            xp = xp_pool.tile([P, HP, WP], BF16, name=f"xp_{s}_{ic}", tag="xp")
            xp_tiles.setdefault(s, {})[ic] = xp
            emit_xp_borders(s, ic, xp)
            return xp

        def emit_xp_interior(s, ic, xss, half, eng):
            xp = xp_tiles[s][ic]
            copy = eng.copy if eng is nc.scalar else eng.tensor_copy
            if half == 0:
                copy(xp[:, 1:RH + 1, 1:WP - 1],
                     xss[ic][:, :RH * W].rearrange("p (h w) -> p h w", w=W))
            else:
                copy(xp[:, RH + 1:HP - 1, 1:WP - 1],
                     xss[ic][:, RH * W:].rearrange("p (h w) -> p h w", w=W))

        def emit_wt_alloc(s):
            mwT[s] = wt_pool.tile([P, NST * P], BF16, name=f"mwT_{s}",
                                  tag="mwT")

        def emit_tp(s, oc, ic):
            # one xbar DMA transpose turns mw[oc]'s ic-half (k-major layout)
            # into 9 [i, o] stationary tiles, written straight into the
            # stationary store; runs on the act ring's DMA queue
            blk = oc * NI + ic
            dst = mwT[s][:, blk * KSQ * P:(blk + 1) * KSQ * P].rearrange(
                "p (k o) -> p k o", o=P)
            # act ring; each DMA ring is a serial FIFO in dispatch order, so
            # the caller interleaves these between x-load dispatches to put
            # them on the wire exactly when needed
            nc.scalar.dma_start(
                out=dst, in_=mw_tiles[s][oc][:, ic * HALF:(ic + 1) * HALF],
                transpose=True)

        def emit_sample_compute(s, interleave):
            """36 weight stages; 7 matmuls each into per-rowgroup PSUM accs.

            interleave: {stage_idx: [callable]} emitted after that stage.
            """
            accs = None
            for t in range(NST):
                oc, ic, kh, kw = stage_params(t)
                local = t % (NI * KSQ)
                last = local == NI * KSQ - 1
                if local == 0:
                    accs = [acc_psum.tile([P, NT], F32,
                                          name=f"acc_{s}_{oc}_{rg}", tag="acc")
                            for rg in range(NRG)]
                if last:
                    osb = osb_pool.tile([P, H * W], F32, name=f"osb_{s}_{oc}",
                                        tag="osb")
                for rg in range(NRG):
                    r0 = rg * RG + kh
                    rhs = xp_tiles[s][ic][:, r0:r0 + RG, kw:kw + W]
                    nc.tensor.matmul(accs[rg], mwT[s][:, t * P:(t + 1) * P],
                                     rhs, start=(local == 0), stop=last)
                    if last:
                        # drain each row-group as soon as its accumulation
                        # stops so the tail doesn't serialize after the PE
                        sl = slice(rg * NT, (rg + 1) * NT)
                        nc.scalar.add(osb[:, sl], accs[rg],
                                      bias_t[:, oc:oc + 1])
                        nc.sync.dma_start(out=o_nat[s, oc][:, sl],
                                          in_=osb[:, sl])
                for fn in interleave.get(t, []):
                    fn()

        # ---------------- prologue: sample 0 ----------------
        # All of sample 0's loads go on the sync ring in the exact order the
        # compute pipeline consumes them: (w+m oc0 first half) -> first mul +
        # transposes, x ic0 -> first matmul stage, then the rest.
        mts0 = [m_pool.tile([P, CFREE], F32, name=f"m_0_{oc}", tag="m")
                for oc in range(NO)]
        xss0 = [xs_pool.tile([P, H * W], F32, name=f"xs_0_{ic}", tag="xs")
                for ic in range(NI)]

        def _m_load(oc, h):
            sl = slice(h * HALF, (h + 1) * HALF)
            nc.sync.dma_start(out=mts0[oc][:, sl], in_=m_nat[0, oc][:, sl])

        def _x_load(ic, h):
            sl = slice(h * RH * W, (h + 1) * RH * W)
            nc.scalar.dma_start(out=xss0[ic][:, sl],
                                in_=x_nat[0, ic][:, h * RH:(h + 1) * RH, :])

        # Each DMA ring is a serial FIFO in dispatch order.  Critical path:
        # sync carries weights+masks in need order, act carries x with the
        # xbar transposes slotted between x chunks, gpsimd (slow SWDGE)
        # carries the late-needed oc1 weights.
        emit_w_load(0, 0)                         # sync
        _m_load(0, 0)                             # sync
        _x_load(0, 0)                             # act
        _x_load(0, 1)                             # act
        emit_w_load(0, 1)                         # sync
        _m_load(0, 1)                             # sync
        emit_w_load(1, 0, nc.gpsimd)              # SWDGE, needed ~t+20us
        emit_w_load(1, 1, nc.gpsimd)
        _m_load(1, 0)                             # sync
        _m_load(1, 1)                             # sync
        nc.sync.dma_start(out=bias_t, in_=b_d.rearrange("(c p) -> p c", p=P))

        emit_mul_alloc(0, 0)
        emit_mul_alloc(0, 1)
        emit_wt_alloc(0)
        emit_xp_alloc(0, 0)
        emit_xp_alloc(0, 1)
        emit_mul_half(0, mts0, 0, 0)              # DVE: mw[0][oc0] ic0 half
        emit_tp(0, 0, 0)                          # act ring: xbar transpose
        _x_load(1, 0)                             # act, behind tp(0,0,0)
        _x_load(1, 1)
        emit_xp_interior(0, 0, xss0, 0, nc.vector)
        emit_xp_interior(0, 0, xss0, 1, nc.vector)
        emit_mul_half(0, mts0, 0, 1)
        emit_tp(0, 0, 1)
        emit_xp_interior(0, 1, xss0, 0, nc.scalar)
        emit_xp_interior(0, 1, xss0, 1, nc.scalar)
        emit_mul_half(0, mts0, 1, 0)
        emit_tp(0, 1, 0)
        emit_mul_half(0, mts0, 1, 1)
        emit_tp(0, 1, 1)

        for s in range(BPC):
            nxt = s + 1
            if nxt < BPC:
                # next-sample loads + weight production, emitted at s's top;
                # everything lands well before s+1's matmuls consume it
                mts = emit_m_loads(nxt)
                xss = emit_x_loads(nxt)
                emit_mul_alloc(nxt, 0)
                emit_mul_alloc(nxt, 1)
                emit_wt_alloc(nxt)
                emit_xp_alloc(nxt, 0)
                emit_xp_alloc(nxt, 1)
                emit_mul_half(nxt, mts, 0, 0)
                emit_tp(nxt, 0, 0)
                emit_xp_interior(nxt, 0, xss, 0, nc.vector)
                emit_xp_interior(nxt, 0, xss, 1, nc.vector)
                emit_mul_half(nxt, mts, 0, 1)
                emit_tp(nxt, 0, 1)
                emit_xp_interior(nxt, 1, xss, 0, nc.scalar)
                emit_xp_interior(nxt, 1, xss, 1, nc.scalar)
                emit_mul_half(nxt, mts, 1, 0)
                emit_tp(nxt, 1, 0)
                emit_mul_half(nxt, mts, 1, 1)
                emit_tp(nxt, 1, 1)
            emit_sample_compute(s, {})

    nc.compile()
    return nc


def shard_inputs(x, m, weight, bias):
    """Split batch across cores; replicate weight/bias."""
    x = np.ascontiguousarray(np.asarray(x, dtype=np.float32))
    m = np.ascontiguousarray(np.asarray(m, dtype=np.float32))
    weight = np.ascontiguousarray(np.asarray(weight, dtype=np.float32))
    bias = np.ascontiguousarray(np.asarray(bias, dtype=np.float32))
    in_maps = []
    for c in range(N_CORES):
        sl = slice(c * BPC, (c + 1) * BPC)
        in_maps.append({"x": x[sl], "m": m[sl], "weight": weight, "bias": bias})
    return in_maps


def kernel(x, m, weight, bias, _trace=False):
    nc = build_program()
    in_maps = shard_inputs(x, m, weight, bias)
    res = bass_utils.run_bass_kernel_spmd(
        nc, in_maps, core_ids=list(range(N_CORES)), trace=_trace
    )
    out = np.concatenate([res.results[c]["out"] for c in range(N_CORES)], axis=0)
    if _trace:
        kernel.last_results = res
    return out


# revision 22
# speedup vs baseline: 1.0848x; 1.0848x over previous
"""Trainium2 Bass kernel for per-sample masked conv2d (dynamic weight attention conv).

out[b] = conv2d(x[b], weight * m[b], stride=1, pad=1) + bias

Strategy: pure data parallel over batch (32 samples -> 8 cores x 4 samples).
Per sample the conv runs as 18 accumulation stages (2 input-channel chunks x 9
taps) of matmuls over 7 row-group PSUM accumulators, so each stationary weight
load serves 7 consecutive matmuls.  The datapath is bf16: masked weights are
built by a DVE multiply (f32 m x f32 w -> bf16), transposed 128x128-tile-wise
on the TensorEngine into [i, o] stationary layout, packed 8-per-PSUM-bank and
drained by the Activation engine.  Transposes for sample s+1 are interleaved
between sample s's matmul stages so the PE never idles.
"""

import sys
from contextlib import ExitStack

for _p in ("/opt/trn_rl_repo",):
    if _p not in sys.path:
        sys.path.append(_p)

import numpy as np

import concourse.bass as bass
import concourse.mybir as mybir
import concourse.tile as tile
from concourse import bacc, bass_utils
from concourse.masks import make_identity

# NOTE: walrus --enable-ldw-opt rejects the standalone InstLdweights that
# bass emits for non-f32 stationary dtypes, so it stays at its default
# (false) for this bf16 kernel.

# Problem constants (hardcoded per contract)
B, FIN, FOUT, KK, H, W = 32, 256, 256, 3, 56, 56
N_CORES = 8
BPC = B // N_CORES          # samples per core = 4
P = 128                     # partition width
NI = FIN // P               # input-channel chunks = 2
NO = FOUT // P              # output-channel chunks = 2
HP, WP = H + 2, W + 2       # padded spatial = 58x58
RG = 8                      # output rows per row-group
NRG = H // RG               # row groups = 7
NT = RG * W                 # matmul moving free size = 448
KSQ = KK * KK               # 9
CFREE = FIN * KSQ           # 2304
HALF = CFREE // NI          # 1152
NST = NO * NI * KSQ         # 36 weight stages per sample
RH = H // 2                 # 28
F32 = mybir.dt.float32
BF16 = mybir.dt.bfloat16

# transpose groups: stages [t0, t1) packed into one PSUM bank per group
TP_GROUPS = [(0, 8), (8, 16), (16, 24), (24, 32), (32, 36)]


def build_program():
    nc = bacc.Bacc("TRN2", target_bir_lowering=False, debug=False,
                   num_devices=N_CORES)

    x_d = nc.dram_tensor("x", [BPC, FIN, H, W], F32, kind="ExternalInput").ap()
    m_d = nc.dram_tensor("m", [BPC, FOUT, FIN, KK, KK], F32,
                         kind="ExternalInput").ap()
    w_d = nc.dram_tensor("weight", [FOUT, FIN, KK, KK], F32,
                         kind="ExternalInput").ap()
    b_d = nc.dram_tensor("bias", [FOUT], F32, kind="ExternalInput").ap()
    o_d = nc.dram_tensor("out", [BPC, FOUT, H, W], F32,
                         kind="ExternalOutput").ap()

    x_nat = x_d.rearrange("s (c p) h w -> s c p h w", p=P)
    m_nat = m_d.rearrange("s (c p) i kh kw -> s c p (i kh kw)", p=P)
    w_nat = w_d.rearrange("(c p) i kh kw -> c p (i kh kw)", p=P)
    o_nat = o_d.rearrange("s (c p) h w -> s c p (h w)", p=P)

    with tile.TileContext(nc) as tc, ExitStack() as ctx:
        consts = ctx.enter_context(tc.tile_pool(name="consts", bufs=1))
        m_pool = ctx.enter_context(tc.tile_pool(name="m_pool", bufs=2))
        mw_pool = ctx.enter_context(tc.tile_pool(name="mw_pool", bufs=2))
        xs_pool = ctx.enter_context(tc.tile_pool(name="xs_pool", bufs=2))
        xp_pool = ctx.enter_context(tc.tile_pool(name="xp_pool", bufs=2 * NI))
        wt_pool = ctx.enter_context(tc.tile_pool(name="wt_pool", bufs=2))
        osb_pool = ctx.enter_context(tc.tile_pool(name="osb_pool", bufs=2))
        acc_psum = ctx.enter_context(tc.tile_pool(name="acc_psum", bufs=NRG,
                                                  space="PSUM"))
        tp_psum = ctx.enter_context(tc.tile_pool(name="tp_psum", bufs=1,
                                                 space="PSUM"))

        ident = consts.tile([P, P], F32, name="ident")
        make_identity(nc, ident)
        ident_b = consts.tile([P, P], BF16, name="ident_b")
        nc.vector.tensor_copy(ident_b, ident)

        # weight tiles, natural layout f32 [oc][128, (i kh kw)]; DMAs are
        # emitted in the prologue interleaved with sample 0's m/x loads on
        # the sync ring, in the exact order the compute consumes them (the
        # DMA engines fair-share bandwidth across outstanding transfers, so
        # issue order controls completion order of the critical first chunks)
        w_tiles = [consts.tile([P, CFREE], F32, name=f"w_{oc}")
                   for oc in range(NO)]
        bias_t = consts.tile([P, NO], F32, name="bias_t")

        def emit_w_load(oc, h, eng=None):
            sl = slice(h * HALF, (h + 1) * HALF)
            (eng or nc.sync).dma_start(out=w_tiles[oc][:, sl],
                                       in_=w_nat[oc][:, sl])

        # per-sample state
        mw_tiles = {}   # s -> [oc] bf16 [P, CFREE]
        mwT = {}        # s -> bf16 [P, NST*P] stationary store
        xp_tiles = {}   # s -> [ic] bf16 [P, HP, WP]

        def stage_params(t):
            oc, r = divmod(t, NI * KSQ)
            ic, k = divmod(r, KSQ)
            kh, kw = divmod(k, KK)
            return oc, ic, kh, kw

        def emit_m_loads(s, eng=None):
            eng = eng or nc.sync
            mts = []
            for oc in range(NO):
                mt = m_pool.tile([P, CFREE], F32, name=f"m_{s}_{oc}", tag="m")
                for h in range(NI):
                    sl = slice(h * HALF, (h + 1) * HALF)
                    eng.dma_start(out=mt[:, sl], in_=m_nat[s, oc][:, sl])
                mts.append(mt)
            return mts

        def emit_x_loads(s, eng=None):
            eng = eng or nc.scalar
            xss = []
            for ic in range(NI):
                xs = xs_pool.tile([P, H * W], F32, name=f"xs_{s}_{ic}",
                                  tag="xs")
                eng.dma_start(out=xs[:, :RH * W], in_=x_nat[s, ic][:, :RH, :])
                eng.dma_start(out=xs[:, RH * W:], in_=x_nat[s, ic][:, RH:, :])
                xss.append(xs)
            return xss

        def emit_mul_alloc(s, oc):
            mw = mw_pool.tile([P, CFREE], BF16, name=f"mw_{s}_{oc}", tag="mw")
            mw_tiles.setdefault(s, []).append(mw)
            return mw

        def emit_mul_half(s, mts, oc, h):
            mw = mw_tiles[s][oc]
            sl = slice(h * HALF, (h + 1) * HALF)
            nc.vector.tensor_mul(mw[:, sl], mts[oc][:, sl], w_tiles[oc][:, sl])

        def emit_mul(s, mts, oc):
            emit_mul_alloc(s, oc)
            for h in range(NI):
                emit_mul_half(s, mts, oc, h)

        def emit_xp_borders(s, ic, xp):
            # zero halo via gpsimd (idle engine); interior overwritten later
            nc.gpsimd.memset(xp[:, 0, :], 0.0)
            nc.gpsimd.memset(xp[:, HP - 1, :], 0.0)
            nc.gpsimd.memset(xp[:, 1:HP - 1, 0:1], 0.0)
            nc.gpsimd.memset(xp[:, 1:HP - 1, WP - 1:WP], 0.0)

        def emit_xp_alloc(s, ic):
            xp = xp_pool.tile([P, HP, WP], BF16, name=f"xp_{s}_{ic}", tag="xp")
            xp_tiles.setdefault(s, {})[ic] = xp
            emit_xp_borders(s, ic, xp)
            return xp

        def emit_xp_interior(s, ic, xss, half, eng):
            xp = xp_tiles[s][ic]
            copy = eng.copy if eng is nc.scalar else eng.tensor_copy
            if half == 0:
                copy(xp[:, 1:RH + 1, 1:WP - 1],
                     xss[ic][:, :RH * W].rearrange("p (h w) -> p h w", w=W))
            else:
                copy(xp[:, RH + 1:HP - 1, 1:WP - 1],
                     xss[ic][:, RH * W:].rearrange("p (h w) -> p h w", w=W))

        def emit_wt_alloc(s):
            mwT[s] = wt_pool.tile([P, NST * P], BF16, name=f"mwT_{s}",
                                  tag="mwT")

        def emit_tp_group(s, gi):
            t0, t1 = TP_GROUPS[gi]
            n = t1 - t0
            tp = tp_psum.tile([P, 8 * P], BF16, name=f"tp_{s}_{gi}", tag="tp")
            for j, t in enumerate(range(t0, t1)):
                oc, ic, kh, kw = stage_params(t)
                k = kh * KK + kw
                mw3 = mw_tiles[s][oc].rearrange("p (i k) -> p i k", k=KSQ)
                nc.tensor.transpose(tp[:, j * P:(j + 1) * P],
                                    mw3[:, ic * P:(ic + 1) * P, k], ident_b)
            # drain the packed bank to the stationary store on Act
            nc.scalar.copy(mwT[s][:, t0 * P:t1 * P], tp[:, :n * P])

        def emit_sample_compute(s, interleave):
            """36 weight stages; 7 matmuls each into per-rowgroup PSUM accs.

            interleave: {stage_idx: [callable]} emitted after that stage.
            """
            accs = None
            for t in range(NST):
                oc, ic, kh, kw = stage_params(t)
                local = t % (NI * KSQ)
                last = local == NI * KSQ - 1
                if local == 0:
                    accs = [acc_psum.tile([P, NT], F32,
                                          name=f"acc_{s}_{oc}_{rg}", tag="acc")
                            for rg in range(NRG)]
                if last:
                    osb = osb_pool.tile([P, H * W], F32, name=f"osb_{s}_{oc}",
                                        tag="osb")
                for rg in range(NRG):
                    r0 = rg * RG + kh
                    rhs = xp_tiles[s][ic][:, r0:r0 + RG, kw:kw + W]
                    nc.tensor.matmul(accs[rg], mwT[s][:, t * P:(t + 1) * P],
                                     rhs, start=(local == 0), stop=last)
                    if last:
                        # drain each row-group as soon as its accumulation
                        # stops so the tail doesn't serialize after the PE
                        sl = slice(rg * NT, (rg + 1) * NT)
                        nc.scalar.add(osb[:, sl], accs[rg],
                                      bias_t[:, oc:oc + 1])
                        nc.sync.dma_start(out=o_nat[s, oc][:, sl],
                                          in_=osb[:, sl])
                for fn in interleave.get(t, []):
                    fn()

        # ---------------- prologue: sample 0 ----------------
        # All of sample 0's loads go on the sync ring in the exact order the
        # compute pipeline consumes them: (w+m oc0 first half) -> first mul +
        # transposes, x ic0 -> first matmul stage, then the rest.
        mts0 = [m_pool.tile([P, CFREE], F32, name=f"m_0_{oc}", tag="m")
                for oc in range(NO)]
        xss0 = [xs_pool.tile([P, H * W], F32, name=f"xs_0_{ic}", tag="xs")
                for ic in range(NI)]

        def _m_load(oc, h):
            sl = slice(h * HALF, (h + 1) * HALF)
            nc.sync.dma_start(out=mts0[oc][:, sl], in_=m_nat[0, oc][:, sl])

        def _x_load(ic, h):
            sl = slice(h * RH * W, (h + 1) * RH * W)
            nc.scalar.dma_start(out=xss0[ic][:, sl],
                                in_=x_nat[0, ic][:, h * RH:(h + 1) * RH, :])

        # Each DMA ring is a serial FIFO in dispatch order, so per-ring load
        # order must equal consumption order: sync carries w/m-oc0 then
        # m-oc1, act carries x, the slow SWDGE ring carries the late-needed
        # w-oc1 halves.
        emit_w_load(0, 0)
        _m_load(0, 0)
        _x_load(0, 0)
        _x_load(0, 1)
        emit_w_load(0, 1)
        _m_load(0, 1)
        _x_load(1, 0)
        _x_load(1, 1)
        emit_w_load(1, 0, nc.gpsimd)
        emit_w_load(1, 1, nc.gpsimd)
        _m_load(1, 0)
        _m_load(1, 1)
        nc.sync.dma_start(out=bias_t, in_=b_d.rearrange("(c p) -> p c", p=P))

        emit_mul_alloc(0, 0)
        emit_mul_alloc(0, 1)
        emit_mul_half(0, mts0, 0, 0)              # DVE: mw[0][oc0] ic0 half
        emit_wt_alloc(0)
        emit_xp_alloc(0, 0)
        emit_xp_alloc(0, 1)
        emit_tp_group(0, 0)                       # PE tps 0-7, drain on Act
        emit_xp_interior(0, 0, xss0, 0, nc.vector)
        emit_xp_interior(0, 0, xss0, 1, nc.vector)
        emit_mul_half(0, mts0, 0, 1)              # DVE: mw[0][oc0] ic1 half
        emit_mul_half(0, mts0, 1, 0)              # DVE: mw[0][oc1] ic0 half
        emit_mul_half(0, mts0, 1, 1)              # DVE: mw[0][oc1] ic1 half
        emit_xp_interior(0, 1, xss0, 0, nc.scalar)
        emit_xp_interior(0, 1, xss0, 1, nc.scalar)

        pending = {}   # interleave map for the current sample's compute

        def add_il(t, fn):
            pending.setdefault(t, []).append(fn)

        # remaining transpose groups of sample 0 interleave into early stages
        add_il(2, lambda: emit_tp_group(0, 1))
        add_il(8, lambda: emit_tp_group(0, 2))
        add_il(11, lambda: emit_tp_group(0, 3))
        add_il(14, lambda: emit_tp_group(0, 4))

        for s in range(BPC):
            nxt = s + 1
            if nxt < BPC:
                # next-sample loads + weight production, emitted at s's top
                mts = emit_m_loads(nxt)
                xss = emit_x_loads(nxt)
                emit_mul(nxt, mts, 0)
                emit_mul(nxt, mts, 1)
                emit_wt_alloc(nxt)
                emit_xp_alloc(nxt, 0)
                emit_xp_alloc(nxt, 1)
                emit_xp_interior(nxt, 0, xss, 0, nc.vector)
                emit_xp_interior(nxt, 0, xss, 1, nc.vector)
                # transposes of s+1 interleave into s's oc1 stages
                add_il(19, lambda s_=nxt: emit_tp_group(s_, 0))
                add_il(20, lambda s_=nxt, x_=xss: emit_xp_interior(
                    s_, 1, x_, 0, nc.scalar))
                add_il(22, lambda s_=nxt: emit_tp_group(s_, 1))
                add_il(23, lambda s_=nxt, x_=xss: emit_xp_interior(
                    s_, 1, x_, 1, nc.scalar))
                add_il(25, lambda s_=nxt: emit_tp_group(s_, 2))
                add_il(28, lambda s_=nxt: emit_tp_group(s_, 3))
                add_il(31, lambda s_=nxt: emit_tp_group(s_, 4))
            emit_sample_compute(s, pending)
            pending = {}

    nc.compile()
    return nc


def shard_inputs(x, m, weight, bias):
    """Split batch across cores; replicate weight/bias."""
    x = np.ascontiguousarray(np.asarray(x, dtype=np.float32))
    m = np.ascontiguousarray(np.asarray(m, dtype=np.float32))
    weight = np.ascontiguousarray(np.asarray(weight, dtype=np.float32))
    bias = np.ascontiguousarray(np.asarray(bias, dtype=np.float32))
    in_maps = []
    for c in range(N_CORES):
        sl = slice(c * BPC, (c + 1) * BPC)
        in_maps.append({"x": x[sl], "m": m[sl], "weight": weight, "bias": bias})
    return in_maps


def kernel(x, m, weight, bias, _trace=False):
    nc = build_program()
    in_maps = shard_inputs(x, m, weight, bias)
    res = bass_utils.run_bass_kernel_spmd(
        nc, in_maps, core_ids=list(range(N_CORES)), trace=_trace
    )
    out = np.concatenate([res.results[c]["out"] for c in range(N_CORES)], axis=0)
    if _trace:
        kernel.last_results = res
    return out


# revision 23
# speedup vs baseline: 1.0935x; 1.0080x over previous
"""Trainium2 Bass kernel for per-sample masked conv2d (dynamic weight attention conv).

out[b] = conv2d(x[b], weight * m[b], stride=1, pad=1) + bias

Strategy: pure data parallel over batch (32 samples -> 8 cores x 4 samples).
Per sample the conv runs as 18 accumulation stages (2 input-channel chunks x 9
taps) of matmuls over 7 row-group PSUM accumulators, so each stationary weight
load serves 7 consecutive matmuls.  The datapath is bf16: masked weights are
built by a DVE multiply (f32 m x f32 w -> bf16), transposed 128x128-tile-wise
on the TensorEngine into [i, o] stationary layout, packed 8-per-PSUM-bank and
drained by the Activation engine.  Transposes for sample s+1 are interleaved
between sample s's matmul stages so the PE never idles.
"""

import sys
from contextlib import ExitStack

for _p in ("/opt/trn_rl_repo",):
    if _p not in sys.path:
        sys.path.append(_p)

import numpy as np

import concourse.bass as bass
import concourse.mybir as mybir
import concourse.tile as tile
from concourse import bacc, bass_utils
from concourse.masks import make_identity

# NOTE: walrus --enable-ldw-opt rejects the standalone InstLdweights that
# bass emits for non-f32 stationary dtypes, so it stays at its default
# (false) for this bf16 kernel.

# Problem constants (hardcoded per contract)
B, FIN, FOUT, KK, H, W = 32, 256, 256, 3, 56, 56
N_CORES = 8
BPC = B // N_CORES          # samples per core = 4
P = 128                     # partition width
NI = FIN // P               # input-channel chunks = 2
NO = FOUT // P              # output-channel chunks = 2
HP, WP = H + 2, W + 2       # padded spatial = 58x58
RG = 8                      # output rows per row-group
NRG = H // RG               # row groups = 7
NT = RG * W                 # matmul moving free size = 448
KSQ = KK * KK               # 9
CFREE = FIN * KSQ           # 2304
HALF = CFREE // NI          # 1152
NST = NO * NI * KSQ         # 36 weight stages per sample
RH = H // 2                 # 28
F32 = mybir.dt.float32
BF16 = mybir.dt.bfloat16

# transpose groups: stages [t0, t1) packed into one PSUM bank per group
TP_GROUPS = [(0, 8), (8, 16), (16, 24), (24, 32), (32, 36)]


def build_program():
    nc = bacc.Bacc("TRN2", target_bir_lowering=False, debug=False,
                   num_devices=N_CORES)

    x_d = nc.dram_tensor("x", [BPC, FIN, H, W], F32, kind="ExternalInput").ap()
    m_d = nc.dram_tensor("m", [BPC, FOUT, FIN, KK, KK], F32,
                         kind="ExternalInput").ap()
    w_d = nc.dram_tensor("weight", [FOUT, FIN, KK, KK], F32,
                         kind="ExternalInput").ap()
    b_d = nc.dram_tensor("bias", [FOUT], F32, kind="ExternalInput").ap()
    o_d = nc.dram_tensor("out", [BPC, FOUT, H, W], F32,
                         kind="ExternalOutput").ap()

    x_nat = x_d.rearrange("s (c p) h w -> s c p h w", p=P)
    m_nat = m_d.rearrange("s (c p) i kh kw -> s c p (i kh kw)", p=P)
    w_nat = w_d.rearrange("(c p) i kh kw -> c p (i kh kw)", p=P)
    o_nat = o_d.rearrange("s (c p) h w -> s c p (h w)", p=P)

    with tile.TileContext(nc) as tc, ExitStack() as ctx:
        consts = ctx.enter_context(tc.tile_pool(name="consts", bufs=1))
        m_pool = ctx.enter_context(tc.tile_pool(name="m_pool", bufs=2))
        mw_pool = ctx.enter_context(tc.tile_pool(name="mw_pool", bufs=2))
        xs_pool = ctx.enter_context(tc.tile_pool(name="xs_pool", bufs=2))
        xp_pool = ctx.enter_context(tc.tile_pool(name="xp_pool", bufs=2 * NI))
        wt_pool = ctx.enter_context(tc.tile_pool(name="wt_pool", bufs=2))
        osb_pool = ctx.enter_context(tc.tile_pool(name="osb_pool", bufs=2))
        acc_psum = ctx.enter_context(tc.tile_pool(name="acc_psum", bufs=NRG,
                                                  space="PSUM"))
        tp_psum = ctx.enter_context(tc.tile_pool(name="tp_psum", bufs=1,
                                                 space="PSUM"))

        ident = consts.tile([P, P], F32, name="ident")
        make_identity(nc, ident)
        ident_b = consts.tile([P, P], BF16, name="ident_b")
        nc.vector.tensor_copy(ident_b, ident)

        # weight tiles, natural layout f32 [oc][128, (i kh kw)]; DMAs are
        # emitted in the prologue interleaved with sample 0's m/x loads on
        # the sync ring, in the exact order the compute consumes them (the
        # DMA engines fair-share bandwidth across outstanding transfers, so
        # issue order controls completion order of the critical first chunks)
        w_tiles = [consts.tile([P, CFREE], F32, name=f"w_{oc}")
                   for oc in range(NO)]
        bias_t = consts.tile([P, NO], F32, name="bias_t")

        def emit_w_load(oc, h, eng=None):
            sl = slice(h * HALF, (h + 1) * HALF)
            (eng or nc.sync).dma_start(out=w_tiles[oc][:, sl],
                                       in_=w_nat[oc][:, sl])

        # per-sample state
        mw_tiles = {}   # s -> [oc] bf16 [P, CFREE]
        mwT = {}        # s -> bf16 [P, NST*P] stationary store
        xp_tiles = {}   # s -> [ic] bf16 [P, HP, WP]

        def stage_params(t):
            oc, r = divmod(t, NI * KSQ)
            ic, k = divmod(r, KSQ)
            kh, kw = divmod(k, KK)
            return oc, ic, kh, kw

        def emit_m_loads(s, eng=None):
            eng = eng or nc.sync
            mts = []
            for oc in range(NO):
                mt = m_pool.tile([P, CFREE], F32, name=f"m_{s}_{oc}", tag="m")
                for h in range(NI):
                    sl = slice(h * HALF, (h + 1) * HALF)
                    eng.dma_start(out=mt[:, sl], in_=m_nat[s, oc][:, sl])
                mts.append(mt)
            return mts

        def emit_x_loads(s, eng=None):
            eng = eng or nc.scalar
            xss = []
            for ic in range(NI):
                xs = xs_pool.tile([P, H * W], F32, name=f"xs_{s}_{ic}",
                                  tag="xs")
                eng.dma_start(out=xs[:, :RH * W], in_=x_nat[s, ic][:, :RH, :])
                eng.dma_start(out=xs[:, RH * W:], in_=x_nat[s, ic][:, RH:, :])
                xss.append(xs)
            return xss

        def emit_mul_alloc(s, oc):
            mw = mw_pool.tile([P, CFREE], BF16, name=f"mw_{s}_{oc}", tag="mw")
            mw_tiles.setdefault(s, []).append(mw)
            return mw

        def emit_mul_half(s, mts, oc, h):
            mw = mw_tiles[s][oc]
            sl = slice(h * HALF, (h + 1) * HALF)
            nc.vector.tensor_mul(mw[:, sl], mts[oc][:, sl], w_tiles[oc][:, sl])

        def emit_mul(s, mts, oc):
            emit_mul_alloc(s, oc)
            for h in range(NI):
                emit_mul_half(s, mts, oc, h)

        def emit_xp_borders(s, ic, xp):
            # zero halo via gpsimd (idle engine); interior overwritten later
            nc.gpsimd.memset(xp[:, 0, :], 0.0)
            nc.gpsimd.memset(xp[:, HP - 1, :], 0.0)
            nc.gpsimd.memset(xp[:, 1:HP - 1, 0:1], 0.0)
            nc.gpsimd.memset(xp[:, 1:HP - 1, WP - 1:WP], 0.0)

        def emit_xp_alloc(s, ic):
            xp = xp_pool.tile([P, HP, WP], BF16, name=f"xp_{s}_{ic}", tag="xp")
            xp_tiles.setdefault(s, {})[ic] = xp
            emit_xp_borders(s, ic, xp)
            return xp

        def emit_xp_interior(s, ic, xss, half, eng):
            xp = xp_tiles[s][ic]
            copy = eng.copy if eng is nc.scalar else eng.tensor_copy
            if half == 0:
                copy(xp[:, 1:RH + 1, 1:WP - 1],
                     xss[ic][:, :RH * W].rearrange("p (h w) -> p h w", w=W))
            else:
                copy(xp[:, RH + 1:HP - 1, 1:WP - 1],
                     xss[ic][:, RH * W:].rearrange("p (h w) -> p h w", w=W))

        def emit_wt_alloc(s):
            mwT[s] = wt_pool.tile([P, NST * P], BF16, name=f"mwT_{s}",
                                  tag="mwT")

        def emit_tp_group(s, gi):
            t0, t1 = TP_GROUPS[gi]
            n = t1 - t0
            tp = tp_psum.tile([P, 8 * P], BF16, name=f"tp_{s}_{gi}", tag="tp")
            for j, t in enumerate(range(t0, t1)):
                oc, ic, kh, kw = stage_params(t)
                k = kh * KK + kw
                mw3 = mw_tiles[s][oc].rearrange("p (i k) -> p i k", k=KSQ)
                nc.tensor.transpose(tp[:, j * P:(j + 1) * P],
                                    mw3[:, ic * P:(ic + 1) * P, k], ident_b)
            # drain the packed bank to the stationary store on Act
            nc.scalar.copy(mwT[s][:, t0 * P:t1 * P], tp[:, :n * P])

        def emit_sample_compute(s, interleave):
            """36 weight stages; 7 matmuls each into per-rowgroup PSUM accs.

            interleave: {stage_idx: [callable]} emitted after that stage.
            """
            accs = None
            for t in range(NST):
                oc, ic, kh, kw = stage_params(t)
                local = t % (NI * KSQ)
                last = local == NI * KSQ - 1
                if local == 0:
                    accs = [acc_psum.tile([P, NT], F32,
                                          name=f"acc_{s}_{oc}_{rg}", tag="acc")
                            for rg in range(NRG)]
                if last:
                    osb = osb_pool.tile([P, H * W], F32, name=f"osb_{s}_{oc}",
                                        tag="osb")
                for rg in range(NRG):
                    r0 = rg * RG + kh
                    rhs = xp_tiles[s][ic][:, r0:r0 + RG, kw:kw + W]
                    nc.tensor.matmul(accs[rg], mwT[s][:, t * P:(t + 1) * P],
                                     rhs, start=(local == 0), stop=last)
                    if last:
                        # drain each row-group as soon as its accumulation
                        # stops so the tail doesn't serialize after the PE
                        sl = slice(rg * NT, (rg + 1) * NT)
                        nc.scalar.add(osb[:, sl], accs[rg],
                                      bias_t[:, oc:oc + 1])
                        nc.sync.dma_start(out=o_nat[s, oc][:, sl],
                                          in_=osb[:, sl])
                for fn in interleave.get(t, []):
                    fn()

        # ---------------- prologue: sample 0 ----------------
        # All of sample 0's loads go on the sync ring in the exact order the
        # compute pipeline consumes them: (w+m oc0 first half) -> first mul +
        # transposes, x ic0 -> first matmul stage, then the rest.
        mts0 = [m_pool.tile([P, CFREE], F32, name=f"m_0_{oc}", tag="m")
                for oc in range(NO)]
        xss0 = [xs_pool.tile([P, H * W], F32, name=f"xs_0_{ic}", tag="xs")
                for ic in range(NI)]

        def _m_load(oc, h):
            sl = slice(h * HALF, (h + 1) * HALF)
            nc.sync.dma_start(out=mts0[oc][:, sl], in_=m_nat[0, oc][:, sl])

        def _x_load(ic, h):
            sl = slice(h * RH * W, (h + 1) * RH * W)
            nc.scalar.dma_start(out=xss0[ic][:, sl],
                                in_=x_nat[0, ic][:, h * RH:(h + 1) * RH, :])

        # Each DMA ring is a serial FIFO in dispatch order, so per-ring load
        # order must equal consumption order: sync carries w/m-oc0 then
        # m-oc1, act carries x, the slow SWDGE ring carries the late-needed
        # w-oc1 halves.
        emit_w_load(0, 0)
        _m_load(0, 0)
        _x_load(0, 0)
        _x_load(0, 1)
        emit_w_load(0, 1)
        _m_load(0, 1)
        _x_load(1, 0)
        _x_load(1, 1)
        emit_w_load(1, 0)
        _m_load(1, 0)
        emit_w_load(1, 1)
        _m_load(1, 1)
        nc.sync.dma_start(out=bias_t, in_=b_d.rearrange("(c p) -> p c", p=P))

        emit_mul_alloc(0, 0)
        emit_mul_alloc(0, 1)
        emit_mul_half(0, mts0, 0, 0)              # DVE: mw[0][oc0] ic0 half
        emit_wt_alloc(0)
        emit_xp_alloc(0, 0)
        emit_xp_alloc(0, 1)
        emit_tp_group(0, 0)                       # PE tps 0-7, drain on Act
        emit_xp_interior(0, 0, xss0, 0, nc.vector)
        emit_xp_interior(0, 0, xss0, 1, nc.vector)
        emit_mul_half(0, mts0, 0, 1)              # DVE: mw[0][oc0] ic1 half
        emit_mul_half(0, mts0, 1, 0)              # DVE: mw[0][oc1] ic0 half
        emit_mul_half(0, mts0, 1, 1)              # DVE: mw[0][oc1] ic1 half
        emit_xp_interior(0, 1, xss0, 0, nc.scalar)
        emit_xp_interior(0, 1, xss0, 1, nc.scalar)

        pending = {}   # interleave map for the current sample's compute

        def add_il(t, fn):
            pending.setdefault(t, []).append(fn)

        # remaining transpose groups of sample 0 interleave into early stages
        add_il(2, lambda: emit_tp_group(0, 1))
        add_il(13, lambda: emit_tp_group(0, 2))
        add_il(20, lambda: emit_tp_group(0, 3))
        add_il(27, lambda: emit_tp_group(0, 4))

        for s in range(BPC):
            nxt = s + 1
            if nxt < BPC:
                # next-sample loads + weight production, emitted at s's top
                mts = emit_m_loads(nxt)
                xss = emit_x_loads(nxt)
                emit_mul(nxt, mts, 0)
                emit_mul(nxt, mts, 1)
                emit_wt_alloc(nxt)
                emit_xp_alloc(nxt, 0)
                emit_xp_alloc(nxt, 1)
                emit_xp_interior(nxt, 0, xss, 0, nc.vector)
                emit_xp_interior(nxt, 0, xss, 1, nc.vector)
                # transposes of s+1 interleave into s's oc1 stages
                add_il(19, lambda s_=nxt: emit_tp_group(s_, 0))
                add_il(20, lambda s_=nxt, x_=xss: emit_xp_interior(
                    s_, 1, x_, 0, nc.scalar))
                add_il(22, lambda s_=nxt: emit_tp_group(s_, 1))
                add_il(23, lambda s_=nxt, x_=xss: emit_xp_interior(
                    s_, 1, x_, 1, nc.scalar))
                add_il(25, lambda s_=nxt: emit_tp_group(s_, 2))
                add_il(28, lambda s_=nxt: emit_tp_group(s_, 3))
                add_il(31, lambda s_=nxt: emit_tp_group(s_, 4))
            emit_sample_compute(s, pending)
            pending = {}

    nc.compile()
    return nc


def shard_inputs(x, m, weight, bias):
    """Split batch across cores; replicate weight/bias."""
    x = np.ascontiguousarray(np.asarray(x, dtype=np.float32))
    m = np.ascontiguousarray(np.asarray(m, dtype=np.float32))
    weight = np.ascontiguousarray(np.asarray(weight, dtype=np.float32))
    bias = np.ascontiguousarray(np.asarray(bias, dtype=np.float32))
    in_maps = []
    for c in range(N_CORES):
        sl = slice(c * BPC, (c + 1) * BPC)
        in_maps.append({"x": x[sl], "m": m[sl], "weight": weight, "bias": bias})
    return in_maps


def kernel(x, m, weight, bias, _trace=False):
    nc = build_program()
    in_maps = shard_inputs(x, m, weight, bias)
    res = bass_utils.run_bass_kernel_spmd(
        nc, in_maps, core_ids=list(range(N_CORES)), trace=_trace
    )
    out = np.concatenate([res.results[c]["out"] for c in range(N_CORES)], axis=0)
    if _trace:
        kernel.last_results = res
    return out


# revision 33
# speedup vs baseline: 1.0972x; 1.0033x over previous
"""Trainium2 Bass kernel for per-sample masked conv2d (dynamic weight attention conv).

out[b] = conv2d(x[b], weight * m[b], stride=1, pad=1) + bias

Strategy: pure data parallel over batch (32 samples -> 8 cores x 4 samples).
Per sample the conv runs as 18 accumulation stages (2 input-channel chunks x 9
taps) of matmuls over 7 row-group PSUM accumulators, so each stationary weight
load serves 7 consecutive matmuls.  The datapath is bf16: masked weights are
built by a DVE multiply (f32 m x f32 w -> bf16), transposed 128x128-tile-wise
on the TensorEngine into [i, o] stationary layout, packed 8-per-PSUM-bank and
drained by the Activation engine.  Transposes for sample s+1 are interleaved
between sample s's matmul stages so the PE never idles.
"""

import sys
from contextlib import ExitStack

for _p in ("/opt/trn_rl_repo",):
    if _p not in sys.path:
        sys.path.append(_p)

import numpy as np

import concourse.bass as bass
import concourse.mybir as mybir
import concourse.tile as tile
from concourse import bacc, bass_utils
from concourse.masks import make_identity

# NOTE: walrus --enable-ldw-opt rejects the standalone InstLdweights that
# bass emits for non-f32 stationary dtypes, so it stays at its default
# (false) for this bf16 kernel.

# Problem constants (hardcoded per contract)
B, FIN, FOUT, KK, H, W = 32, 256, 256, 3, 56, 56
N_CORES = 8
BPC = B // N_CORES          # samples per core = 4
P = 128                     # partition width
NI = FIN // P               # input-channel chunks = 2
NO = FOUT // P              # output-channel chunks = 2
HP, WP = H + 2, W + 2       # padded spatial = 58x58
RG = 8                      # output rows per row-group
NRG = H // RG               # row groups = 7
NT = RG * W                 # matmul moving free size = 448
KSQ = KK * KK               # 9
CFREE = FIN * KSQ           # 2304
HALF = CFREE // NI          # 1152
NST = NO * NI * KSQ         # 36 weight stages per sample
RH = H // 2                 # 28
F32 = mybir.dt.float32
BF16 = mybir.dt.bfloat16

# transpose groups: stages [t0, t1) packed into one PSUM bank per group
TP_GROUPS = [(0, 8), (8, 16), (16, 24), (24, 32), (32, 36)]


def build_program():
    nc = bacc.Bacc("TRN2", target_bir_lowering=False, debug=False,
                   num_devices=N_CORES)

    x_d = nc.dram_tensor("x", [BPC, FIN, H, W], F32, kind="ExternalInput").ap()
    m_d = nc.dram_tensor("m", [BPC, FOUT, FIN, KK, KK], F32,
                         kind="ExternalInput").ap()
    w_d = nc.dram_tensor("weight", [FOUT, FIN, KK, KK], F32,
                         kind="ExternalInput").ap()
    b_d = nc.dram_tensor("bias", [FOUT], F32, kind="ExternalInput").ap()
    o_d = nc.dram_tensor("out", [BPC, FOUT, H, W], F32,
                         kind="ExternalOutput").ap()

    x_nat = x_d.rearrange("s (c p) h w -> s c p h w", p=P)
    m_nat = m_d.rearrange("s (c p) i kh kw -> s c p (i kh kw)", p=P)
    w_nat = w_d.rearrange("(c p) i kh kw -> c p (i kh kw)", p=P)
    o_nat = o_d.rearrange("s (c p) h w -> s c p (h w)", p=P)

    with tile.TileContext(nc) as tc, ExitStack() as ctx:
        consts = ctx.enter_context(tc.tile_pool(name="consts", bufs=1))
        m_pool = ctx.enter_context(tc.tile_pool(name="m_pool", bufs=2))
        mw_pool = ctx.enter_context(tc.tile_pool(name="mw_pool", bufs=2))
        xs_pool = ctx.enter_context(tc.tile_pool(name="xs_pool", bufs=2))
        xp_pool = ctx.enter_context(tc.tile_pool(name="xp_pool", bufs=2 * NI))
        wt_pool = ctx.enter_context(tc.tile_pool(name="wt_pool", bufs=2))
        osb_pool = ctx.enter_context(tc.tile_pool(name="osb_pool", bufs=2))
        acc_psum = ctx.enter_context(tc.tile_pool(name="acc_psum", bufs=NRG,
                                                  space="PSUM"))
        tp_psum = ctx.enter_context(tc.tile_pool(name="tp_psum", bufs=1,
                                                 space="PSUM"))

        ident = consts.tile([P, P], F32, name="ident")
        make_identity(nc, ident)
        ident_b = consts.tile([P, P], BF16, name="ident_b")
        nc.vector.tensor_copy(ident_b, ident)

        # weight tiles, natural layout f32 [oc][128, (i kh kw)]; DMAs are
        # emitted in the prologue interleaved with sample 0's m loads on the
        # sync ring in the exact order the compute consumes them (each DMA
        # ring is a serial FIFO, so per-ring issue order IS completion order
        # and a bulk transfer ahead of a critical chunk delays it fully)
        w_tiles = [consts.tile([P, CFREE], F32, name=f"w_{oc}")
                   for oc in range(NO)]
        bias_t = consts.tile([P, NO], F32, name="bias_t")

        def emit_w_load(oc, h):
            sl = slice(h * HALF, (h + 1) * HALF)
            nc.sync.dma_start(out=w_tiles[oc][:, sl], in_=w_nat[oc][:, sl])

        # per-sample state
        mw_tiles = {}   # s -> [oc] bf16 [P, CFREE]
        mwT = {}        # s -> bf16 [P, NST*P] stationary store
        xp_tiles = {}   # s -> [ic] bf16 [P, HP, WP]

        def stage_params(t):
            oc, r = divmod(t, NI * KSQ)
            ic, k = divmod(r, KSQ)
            kh, kw = divmod(k, KK)
            return oc, ic, kh, kw

        def emit_m_loads(s, eng=None):
            eng = eng or nc.sync
            mts = []
            for oc in range(NO):
                mt = m_pool.tile([P, CFREE], F32, name=f"m_{s}_{oc}", tag="m")
                for h in range(NI):
                    sl = slice(h * HALF, (h + 1) * HALF)
                    eng.dma_start(out=mt[:, sl], in_=m_nat[s, oc][:, sl])
                mts.append(mt)
            return mts

        def emit_x_loads(s, eng=None):
            eng = eng or nc.scalar
            xss = []
            for ic in range(NI):
                xs = xs_pool.tile([P, H * W], F32, name=f"xs_{s}_{ic}",
                                  tag="xs")
                eng.dma_start(out=xs[:, :RH * W], in_=x_nat[s, ic][:, :RH, :])
                eng.dma_start(out=xs[:, RH * W:], in_=x_nat[s, ic][:, RH:, :])
                xss.append(xs)
            return xss

        def emit_mul_alloc(s, oc):
            mw = mw_pool.tile([P, CFREE], BF16, name=f"mw_{s}_{oc}", tag="mw")
            mw_tiles.setdefault(s, []).append(mw)
            return mw

        def emit_mul_half(s, mts, oc, h):
            mw = mw_tiles[s][oc]
            sl = slice(h * HALF, (h + 1) * HALF)
            nc.vector.tensor_mul(mw[:, sl], mts[oc][:, sl], w_tiles[oc][:, sl])

        def emit_mul(s, mts, oc):
            emit_mul_alloc(s, oc)
            for h in range(NI):
                emit_mul_half(s, mts, oc, h)

        def emit_xp_borders(s, ic, xp):
            # zero halo via gpsimd (idle engine); interior overwritten later
            nc.gpsimd.memset(xp[:, 0, :], 0.0)
            nc.gpsimd.memset(xp[:, HP - 1, :], 0.0)
            nc.gpsimd.memset(xp[:, 1:HP - 1, 0:1], 0.0)
            nc.gpsimd.memset(xp[:, 1:HP - 1, WP - 1:WP], 0.0)

        def emit_xp_alloc(s, ic):
            xp = xp_pool.tile([P, HP, WP], BF16, name=f"xp_{s}_{ic}", tag="xp")
            xp_tiles.setdefault(s, {})[ic] = xp
            emit_xp_borders(s, ic, xp)
            return xp

        def emit_xp_interior(s, ic, xss, half, eng):
            xp = xp_tiles[s][ic]
            copy = eng.copy if eng is nc.scalar else eng.tensor_copy
            if half == 0:
                copy(xp[:, 1:RH + 1, 1:WP - 1],
                     xss[ic][:, :RH * W].rearrange("p (h w) -> p h w", w=W))
            else:
                copy(xp[:, RH + 1:HP - 1, 1:WP - 1],
                     xss[ic][:, RH * W:].rearrange("p (h w) -> p h w", w=W))

        def emit_wt_alloc(s):
            mwT[s] = wt_pool.tile([P, NST * P], BF16, name=f"mwT_{s}",
                                  tag="mwT")

        def emit_tp_group(s, gi):
            t0, t1 = TP_GROUPS[gi]
            n = t1 - t0
            tp = tp_psum.tile([P, 8 * P], BF16, name=f"tp_{s}_{gi}", tag="tp")
            for j, t in enumerate(range(t0, t1)):
                oc, ic, kh, kw = stage_params(t)
                k = kh * KK + kw
                mw3 = mw_tiles[s][oc].rearrange("p (i k) -> p i k", k=KSQ)
                nc.tensor.transpose(tp[:, j * P:(j + 1) * P],
                                    mw3[:, ic * P:(ic + 1) * P, k], ident_b)
            # drain the packed bank to the stationary store on Act
            nc.scalar.copy(mwT[s][:, t0 * P:t1 * P], tp[:, :n * P])

        def emit_sample_compute(s, interleave):
            """36 weight stages; 7 matmuls each into per-rowgroup PSUM accs.

            interleave: {stage_idx: [callable]} emitted after that stage.
            """
            accs = None
            for t in range(NST):
                oc, ic, kh, kw = stage_params(t)
                local = t % (NI * KSQ)
                last = local == NI * KSQ - 1
                if local == 0:
                    accs = [acc_psum.tile([P, NT], F32,
                                          name=f"acc_{s}_{oc}_{rg}", tag="acc")
                            for rg in range(NRG)]
                if last:
                    osb = osb_pool.tile([P, H * W], F32, name=f"osb_{s}_{oc}",
                                        tag="osb")
                for rg in range(NRG):
                    r0 = rg * RG + kh
                    rhs = xp_tiles[s][ic][:, r0:r0 + RG, kw:kw + W]
                    nc.tensor.matmul(accs[rg], mwT[s][:, t * P:(t + 1) * P],
                                     rhs, start=(local == 0), stop=last)
                    if last:
                        # drain each row-group as soon as its accumulation
                        # stops so the tail doesn't serialize after the PE;
                        # on the final oc, alternate Act/DVE (both idle then)
                        # so the 7-drain chain halves
                        sl = slice(rg * NT, (rg + 1) * NT)
                        if s == BPC - 1 and oc == NO - 1 and rg % 2 == 1:
                            nc.vector.tensor_scalar_add(osb[:, sl], accs[rg],
                                                        bias_t[:, oc:oc + 1])
                        else:
                            nc.scalar.add(osb[:, sl], accs[rg],
                                          bias_t[:, oc:oc + 1])
                        # final sample: alternate store rings so the last 7
                        # output DMAs drain two FIFOs instead of one
                        oeng = (nc.scalar if s == BPC - 1 and rg % 2 == 1
                                else nc.sync)
                        oeng.dma_start(out=o_nat[s, oc][:, sl],
                                       in_=osb[:, sl])
                for fn in interleave.get(t, []):
                    fn()

        # ---------------- prologue: sample 0 ----------------
        # All of sample 0's loads go on the sync ring in the exact order the
        # compute pipeline consumes them: (w+m oc0 first half) -> first mul +
        # transposes, x ic0 -> first matmul stage, then the rest.
        mts0 = [m_pool.tile([P, CFREE], F32, name=f"m_0_{oc}", tag="m")
                for oc in range(NO)]
        xss0 = [xs_pool.tile([P, H * W], F32, name=f"xs_0_{ic}", tag="xs")
                for ic in range(NI)]

        def _m_load(oc, h):
            sl = slice(h * HALF, (h + 1) * HALF)
            nc.sync.dma_start(out=mts0[oc][:, sl], in_=m_nat[0, oc][:, sl])

        def _x_load(ic, h):
            sl = slice(h * RH * W, (h + 1) * RH * W)
            nc.scalar.dma_start(out=xss0[ic][:, sl],
                                in_=x_nat[0, ic][:, h * RH:(h + 1) * RH, :])

        # load order == consumption order: weights+masks interleaved on the
        # sync ring (oc1 chunks late, matching their transpose-group slots),
        # x on the act ring, so the first mul/transpose chunks finish first
        emit_w_load(0, 0)
        _m_load(0, 0)
        _x_load(0, 0)
        _x_load(0, 1)
        emit_w_load(0, 1)
        _m_load(0, 1)
        _x_load(1, 0)
        _x_load(1, 1)
        emit_w_load(1, 0)
        _m_load(1, 0)
        emit_w_load(1, 1)
        _m_load(1, 1)
        nc.sync.dma_start(out=bias_t, in_=b_d.rearrange("(c p) -> p c", p=P))

        emit_mul_alloc(0, 0)
        emit_mul_alloc(0, 1)
        emit_mul_half(0, mts0, 0, 0)              # DVE: mw[0][oc0] ic0 half
        emit_wt_alloc(0)
        emit_xp_alloc(0, 0)
        emit_xp_alloc(0, 1)
        emit_tp_group(0, 0)                       # PE tps 0-7, drain on Act
        emit_xp_interior(0, 0, xss0, 0, nc.vector)
        emit_xp_interior(0, 0, xss0, 1, nc.vector)
        emit_mul_half(0, mts0, 0, 1)              # DVE: mw[0][oc0] ic1 half
        emit_mul_half(0, mts0, 1, 0)              # DVE: mw[0][oc1] ic0 half
        emit_mul_half(0, mts0, 1, 1)              # DVE: mw[0][oc1] ic1 half
        emit_xp_interior(0, 1, xss0, 0, nc.scalar)
        emit_xp_interior(0, 1, xss0, 1, nc.scalar)

        pending = {}   # interleave map for the current sample's compute

        def add_il(t, fn):
            pending.setdefault(t, []).append(fn)

        # remaining transpose groups of sample 0 interleave into early stages
        add_il(2, lambda: emit_tp_group(0, 1))
        add_il(14, lambda: emit_tp_group(0, 2))
        add_il(21, lambda: emit_tp_group(0, 3))
        add_il(28, lambda: emit_tp_group(0, 4))

        for s in range(BPC):
            nxt = s + 1
            if nxt < BPC:
                # next-sample loads + weight production, emitted at s's top
                mts = emit_m_loads(nxt)
                xss = emit_x_loads(nxt)
                emit_mul(nxt, mts, 0)
                emit_mul(nxt, mts, 1)
                emit_wt_alloc(nxt)
                emit_xp_alloc(nxt, 0)
                emit_xp_alloc(nxt, 1)
                emit_xp_interior(nxt, 0, xss, 0, nc.vector)
                emit_xp_interior(nxt, 0, xss, 1, nc.vector)
                # transposes of s+1 interleave into s's oc1 stages
                add_il(19, lambda s_=nxt: emit_tp_group(s_, 0))
                add_il(20, lambda s_=nxt, x_=xss: emit_xp_interior(
                    s_, 1, x_, 0, nc.scalar))
                add_il(22, lambda s_=nxt: emit_tp_group(s_, 1))
                add_il(23, lambda s_=nxt, x_=xss: emit_xp_interior(
                    s_, 1, x_, 1, nc.scalar))
                add_il(25, lambda s_=nxt: emit_tp_group(s_, 2))
                add_il(28, lambda s_=nxt: emit_tp_group(s_, 3))
                add_il(31, lambda s_=nxt: emit_tp_group(s_, 4))
            emit_sample_compute(s, pending)
            pending = {}

    nc.compile()
    return nc


def shard_inputs(x, m, weight, bias):
    """Split batch across cores; replicate weight/bias."""
    x = np.ascontiguousarray(np.asarray(x, dtype=np.float32))
    m = np.ascontiguousarray(np.asarray(m, dtype=np.float32))
    weight = np.ascontiguousarray(np.asarray(weight, dtype=np.float32))
    bias = np.ascontiguousarray(np.asarray(bias, dtype=np.float32))
    in_maps = []
    for c in range(N_CORES):
        sl = slice(c * BPC, (c + 1) * BPC)
        in_maps.append({"x": x[sl], "m": m[sl], "weight": weight, "bias": bias})
    return in_maps


def kernel(x, m, weight, bias, _trace=False):
    nc = build_program()
    in_maps = shard_inputs(x, m, weight, bias)
    try:
        res = bass_utils.run_bass_kernel_spmd(
            nc, in_maps, core_ids=list(range(N_CORES)), trace=_trace
        )
    except Exception:
        # sporadic NRT_EXEC_UNIT_UNRECOVERABLE transients recover on retry
        res = bass_utils.run_bass_kernel_spmd(
            nc, in_maps, core_ids=list(range(N_CORES)), trace=_trace
        )
    out = np.concatenate([res.results[c]["out"] for c in range(N_CORES)], axis=0)
    if _trace:
        kernel.last_results = res
    return out


# revision 34
# speedup vs baseline: 1.1246x; 1.0250x over previous
"""Trainium2 Bass kernel for per-sample masked conv2d (dynamic weight attention conv).

out[b] = conv2d(x[b], weight * m[b], stride=1, pad=1) + bias

Strategy: pure data parallel over batch (32 samples -> 8 cores x 4 samples).
Per sample the conv runs as 18 accumulation stages (2 input-channel chunks x 9
taps) of matmuls over 7 row-group PSUM accumulators, so each stationary weight
load serves 7 consecutive matmuls.  The datapath is bf16: masked weights are
built by a DVE multiply (f32 m x f32 w -> bf16), transposed 128x128-tile-wise
on the TensorEngine into [i, o] stationary layout, packed 8-per-PSUM-bank and
drained by the Activation engine.  Transposes for sample s+1 are interleaved
between sample s's matmul stages so the PE never idles.
"""

import sys
from contextlib import ExitStack

for _p in ("/opt/trn_rl_repo",):
    if _p not in sys.path:
        sys.path.append(_p)

import numpy as np

import concourse.bass as bass
import concourse.mybir as mybir
import concourse.tile as tile
from concourse import bacc, bass_utils
from concourse.masks import make_identity

# NOTE: walrus --enable-ldw-opt rejects the standalone InstLdweights that
# bass emits for non-f32 stationary dtypes, so it stays at its default
# (false) for this bf16 kernel.

# Problem constants (hardcoded per contract)
B, FIN, FOUT, KK, H, W = 32, 256, 256, 3, 56, 56
N_CORES = 8
BPC = B // N_CORES          # samples per core = 4
P = 128                     # partition width
NI = FIN // P               # input-channel chunks = 2
NO = FOUT // P              # output-channel chunks = 2
HP, WP = H + 2, W + 2       # padded spatial = 58x58
RG = 8                      # output rows per row-group
NRG = H // RG               # row groups = 7
NT = RG * W                 # matmul moving free size = 448
KSQ = KK * KK               # 9
CFREE = FIN * KSQ           # 2304
HALF = CFREE // NI          # 1152
NST = NO * NI * KSQ         # 36 weight stages per sample
RH = H // 2                 # 28
F32 = mybir.dt.float32
BF16 = mybir.dt.bfloat16

# transpose groups: stages [t0, t1) packed into one PSUM bank per group
TP_GROUPS = [(0, 8), (8, 16), (16, 24), (24, 32), (32, 36)]


def build_program():
    nc = bacc.Bacc("TRN2", target_bir_lowering=False, debug=False,
                   num_devices=N_CORES)

    x_d = nc.dram_tensor("x", [BPC, FIN, H, W], F32, kind="ExternalInput").ap()
    m_d = nc.dram_tensor("m", [BPC, FOUT, FIN, KK, KK], F32,
                         kind="ExternalInput").ap()
    w_d = nc.dram_tensor("weight", [FOUT, FIN, KK, KK], F32,
                         kind="ExternalInput").ap()
    b_d = nc.dram_tensor("bias", [FOUT], F32, kind="ExternalInput").ap()
    o_d = nc.dram_tensor("out", [BPC, FOUT, H, W], F32,
                         kind="ExternalOutput").ap()

    x_nat = x_d.rearrange("s (c p) h w -> s c p h w", p=P)
    m_nat = m_d.rearrange("s (c p) i kh kw -> s c p (i kh kw)", p=P)
    w_nat = w_d.rearrange("(c p) i kh kw -> c p (i kh kw)", p=P)
    o_nat = o_d.rearrange("s (c p) h w -> s c p (h w)", p=P)

    with tile.TileContext(nc) as tc, ExitStack() as ctx:
        consts = ctx.enter_context(tc.tile_pool(name="consts", bufs=1))
        m_pool = ctx.enter_context(tc.tile_pool(name="m_pool", bufs=2))
        mw_pool = ctx.enter_context(tc.tile_pool(name="mw_pool", bufs=2))
        xs_pool = ctx.enter_context(tc.tile_pool(name="xs_pool", bufs=2))
        xp_pool = ctx.enter_context(tc.tile_pool(name="xp_pool", bufs=2 * NI))
        wt_pool = ctx.enter_context(tc.tile_pool(name="wt_pool", bufs=2))
        osb_pool = ctx.enter_context(tc.tile_pool(name="osb_pool", bufs=2))
        acc_psum = ctx.enter_context(tc.tile_pool(name="acc_psum", bufs=NRG,
                                                  space="PSUM"))
        tp_psum = ctx.enter_context(tc.tile_pool(name="tp_psum", bufs=1,
                                                 space="PSUM"))

        ident = consts.tile([P, P], F32, name="ident")
        make_identity(nc, ident)
        ident_b = consts.tile([P, P], BF16, name="ident_b")
        nc.vector.tensor_copy(ident_b, ident)

        # weight tiles, natural layout f32 [oc][128, (i kh kw)]; DMAs are
        # emitted in the prologue interleaved with sample 0's m loads on the
        # sync ring in the exact order the compute consumes them (each DMA
        # ring is a serial FIFO, so per-ring issue order IS completion order
        # and a bulk transfer ahead of a critical chunk delays it fully)
        w_tiles = [consts.tile([P, CFREE], F32, name=f"w_{oc}")
                   for oc in range(NO)]
        bias_t = consts.tile([P, NO], F32, name="bias_t")

        def emit_w_load(oc, h):
            sl = slice(h * HALF, (h + 1) * HALF)
            nc.sync.dma_start(out=w_tiles[oc][:, sl], in_=w_nat[oc][:, sl])

        # per-sample state
        mw_tiles = {}   # s -> [oc] bf16 [P, CFREE]
        mwT = {}        # s -> bf16 [P, NST*P] stationary store
        xp_tiles = {}   # s -> [ic] bf16 [P, HP, WP]

        def stage_params(t):
            oc, r = divmod(t, NI * KSQ)
            ic, k = divmod(r, KSQ)
            kh, kw = divmod(k, KK)
            return oc, ic, kh, kw

        def emit_m_loads(s, eng=None):
            eng = eng or nc.sync
            mts = []
            for oc in range(NO):
                mt = m_pool.tile([P, CFREE], F32, name=f"m_{s}_{oc}", tag="m")
                for h in range(NI):
                    sl = slice(h * HALF, (h + 1) * HALF)
                    eng.dma_start(out=mt[:, sl], in_=m_nat[s, oc][:, sl])
                mts.append(mt)
            return mts

        def emit_x_loads(s, eng=None):
            eng = eng or nc.scalar
            xss = []
            for ic in range(NI):
                xs = xs_pool.tile([P, H * W], F32, name=f"xs_{s}_{ic}",
                                  tag="xs")
                eng.dma_start(out=xs[:, :RH * W], in_=x_nat[s, ic][:, :RH, :])
                eng.dma_start(out=xs[:, RH * W:], in_=x_nat[s, ic][:, RH:, :])
                xss.append(xs)
            return xss

        def emit_mul_alloc(s, oc):
            mw = mw_pool.tile([P, CFREE], BF16, name=f"mw_{s}_{oc}", tag="mw")
            mw_tiles.setdefault(s, []).append(mw)
            return mw

        def emit_mul_half(s, mts, oc, h):
            mw = mw_tiles[s][oc]
            sl = slice(h * HALF, (h + 1) * HALF)
            nc.vector.tensor_mul(mw[:, sl], mts[oc][:, sl], w_tiles[oc][:, sl])

        def emit_mul(s, mts, oc):
            emit_mul_alloc(s, oc)
            for h in range(NI):
                emit_mul_half(s, mts, oc, h)

        def emit_xp_borders(s, ic, xp):
            # zero halo via gpsimd (idle engine); interior overwritten later
            nc.gpsimd.memset(xp[:, 0, :], 0.0)
            nc.gpsimd.memset(xp[:, HP - 1, :], 0.0)
            nc.gpsimd.memset(xp[:, 1:HP - 1, 0:1], 0.0)
            nc.gpsimd.memset(xp[:, 1:HP - 1, WP - 1:WP], 0.0)

        def emit_xp_alloc(s, ic):
            xp = xp_pool.tile([P, HP, WP], BF16, name=f"xp_{s}_{ic}", tag="xp")
            xp_tiles.setdefault(s, {})[ic] = xp
            emit_xp_borders(s, ic, xp)
            return xp

        def emit_xp_interior(s, ic, xss, half, eng):
            xp = xp_tiles[s][ic]
            copy = eng.copy if eng is nc.scalar else eng.tensor_copy
            if half == 0:
                copy(xp[:, 1:RH + 1, 1:WP - 1],
                     xss[ic][:, :RH * W].rearrange("p (h w) -> p h w", w=W))
            else:
                copy(xp[:, RH + 1:HP - 1, 1:WP - 1],
                     xss[ic][:, RH * W:].rearrange("p (h w) -> p h w", w=W))

        def emit_wt_alloc(s):
            mwT[s] = wt_pool.tile([P, NST * P], BF16, name=f"mwT_{s}",
                                  tag="mwT")

        def emit_tp_group(s, gi):
            t0, t1 = TP_GROUPS[gi]
            n = t1 - t0
            tp = tp_psum.tile([P, 8 * P], BF16, name=f"tp_{s}_{gi}", tag="tp")
            for j, t in enumerate(range(t0, t1)):
                oc, ic, kh, kw = stage_params(t)
                k = kh * KK + kw
                mw3 = mw_tiles[s][oc].rearrange("p (i k) -> p i k", k=KSQ)
                nc.tensor.transpose(tp[:, j * P:(j + 1) * P],
                                    mw3[:, ic * P:(ic + 1) * P, k], ident_b)
            # drain the packed bank to the stationary store on Act
            nc.scalar.copy(mwT[s][:, t0 * P:t1 * P], tp[:, :n * P])

        def emit_sample_compute(s, interleave):
            """36 weight stages; 7 matmuls each into per-rowgroup PSUM accs.

            interleave: {stage_idx: [callable]} emitted after that stage.
            """
            accs = None
            for t in range(NST):
                oc, ic, kh, kw = stage_params(t)
                local = t % (NI * KSQ)
                last = local == NI * KSQ - 1
                if local == 0:
                    accs = [acc_psum.tile([P, NT], F32,
                                          name=f"acc_{s}_{oc}_{rg}", tag="acc")
                            for rg in range(NRG)]
                if last:
                    osb = osb_pool.tile([P, H * W], F32, name=f"osb_{s}_{oc}",
                                        tag="osb")
                for rg in range(NRG):
                    r0 = rg * RG + kh
                    rhs = xp_tiles[s][ic][:, r0:r0 + RG, kw:kw + W]
                    nc.tensor.matmul(accs[rg], mwT[s][:, t * P:(t + 1) * P],
                                     rhs, start=(local == 0), stop=last)
                    if last:
                        # drain each row-group as soon as its accumulation
                        # stops so the tail doesn't serialize after the PE;
                        # on the final oc, alternate Act/DVE (both idle then)
                        # so the 7-drain chain halves
                        sl = slice(rg * NT, (rg + 1) * NT)
                        if s == BPC - 1 and oc == NO - 1 and rg % 2 == 1:
                            nc.vector.tensor_scalar_add(osb[:, sl], accs[rg],
                                                        bias_t[:, oc:oc + 1])
                        else:
                            nc.scalar.add(osb[:, sl], accs[rg],
                                          bias_t[:, oc:oc + 1])
                        nc.sync.dma_start(out=o_nat[s, oc][:, sl],
                                          in_=osb[:, sl])
                for fn in interleave.get(t, []):
                    fn()

        # ---------------- prologue: sample 0 ----------------
        # All of sample 0's loads go on the sync ring in the exact order the
        # compute pipeline consumes them: (w+m oc0 first half) -> first mul +
        # transposes, x ic0 -> first matmul stage, then the rest.
        mts0 = [m_pool.tile([P, CFREE], F32, name=f"m_0_{oc}", tag="m")
                for oc in range(NO)]
        xss0 = [xs_pool.tile([P, H * W], F32, name=f"xs_0_{ic}", tag="xs")
                for ic in range(NI)]

        def _m_load(oc, h):
            sl = slice(h * HALF, (h + 1) * HALF)
            nc.sync.dma_start(out=mts0[oc][:, sl], in_=m_nat[0, oc][:, sl])

        def _x_load(ic, h):
            sl = slice(h * RH * W, (h + 1) * RH * W)
            nc.scalar.dma_start(out=xss0[ic][:, sl],
                                in_=x_nat[0, ic][:, h * RH:(h + 1) * RH, :])

        # load order == consumption order: weights+masks interleaved on the
        # sync ring (oc1 chunks late, matching their transpose-group slots),
        # x on the act ring, so the first mul/transpose chunks finish first
        emit_w_load(0, 0)
        _m_load(0, 0)
        _x_load(0, 0)
        _x_load(0, 1)
        emit_w_load(0, 1)
        _m_load(0, 1)
        _x_load(1, 0)
        _x_load(1, 1)
        emit_w_load(1, 0)
        _m_load(1, 0)
        emit_w_load(1, 1)
        _m_load(1, 1)
        nc.sync.dma_start(out=bias_t, in_=b_d.rearrange("(c p) -> p c", p=P))

        emit_mul_alloc(0, 0)
        emit_mul_alloc(0, 1)
        emit_mul_half(0, mts0, 0, 0)              # DVE: mw[0][oc0] ic0 half
        emit_wt_alloc(0)
        emit_xp_alloc(0, 0)
        emit_xp_alloc(0, 1)
        emit_tp_group(0, 0)                       # PE tps 0-7, drain on Act
        emit_xp_interior(0, 0, xss0, 0, nc.vector)
        emit_xp_interior(0, 0, xss0, 1, nc.vector)
        emit_mul_half(0, mts0, 0, 1)              # DVE: mw[0][oc0] ic1 half
        emit_mul_half(0, mts0, 1, 0)              # DVE: mw[0][oc1] ic0 half
        emit_mul_half(0, mts0, 1, 1)              # DVE: mw[0][oc1] ic1 half
        emit_xp_interior(0, 1, xss0, 0, nc.scalar)
        emit_xp_interior(0, 1, xss0, 1, nc.scalar)

        pending = {}   # interleave map for the current sample's compute

        def add_il(t, fn):
            pending.setdefault(t, []).append(fn)

        # remaining transpose groups of sample 0 interleave into early stages
        add_il(2, lambda: emit_tp_group(0, 1))
        add_il(14, lambda: emit_tp_group(0, 2))
        add_il(21, lambda: emit_tp_group(0, 3))
        add_il(28, lambda: emit_tp_group(0, 4))

        for s in range(BPC):
            nxt = s + 1
            if nxt < BPC:
                # next-sample loads + weight production, emitted at s's top
                mts = emit_m_loads(nxt)
                xss = emit_x_loads(nxt)
                emit_mul(nxt, mts, 0)
                emit_mul(nxt, mts, 1)
                emit_wt_alloc(nxt)
                emit_xp_alloc(nxt, 0)
                emit_xp_alloc(nxt, 1)
                emit_xp_interior(nxt, 0, xss, 0, nc.vector)
                emit_xp_interior(nxt, 0, xss, 1, nc.vector)
                # transposes of s+1 interleave into s's oc1 stages
                add_il(19, lambda s_=nxt: emit_tp_group(s_, 0))
                add_il(20, lambda s_=nxt, x_=xss: emit_xp_interior(
                    s_, 1, x_, 0, nc.scalar))
                add_il(22, lambda s_=nxt: emit_tp_group(s_, 1))
                add_il(23, lambda s_=nxt, x_=xss: emit_xp_interior(
                    s_, 1, x_, 1, nc.scalar))
                add_il(25, lambda s_=nxt: emit_tp_group(s_, 2))
                add_il(28, lambda s_=nxt: emit_tp_group(s_, 3))
                add_il(31, lambda s_=nxt: emit_tp_group(s_, 4))
            emit_sample_compute(s, pending)
            pending = {}

    nc.compile()
    return nc


def shard_inputs(x, m, weight, bias):
    """Split batch across cores; replicate weight/bias."""
    x = np.ascontiguousarray(np.asarray(x, dtype=np.float32))
    m = np.ascontiguousarray(np.asarray(m, dtype=np.float32))
    weight = np.ascontiguousarray(np.asarray(weight, dtype=np.float32))
    bias = np.ascontiguousarray(np.asarray(bias, dtype=np.float32))
    in_maps = []
    for c in range(N_CORES):
        sl = slice(c * BPC, (c + 1) * BPC)
        in_maps.append({"x": x[sl], "m": m[sl], "weight": weight, "bias": bias})
    return in_maps


def kernel(x, m, weight, bias, _trace=False):
    nc = build_program()
    in_maps = shard_inputs(x, m, weight, bias)
    try:
        res = bass_utils.run_bass_kernel_spmd(
            nc, in_maps, core_ids=list(range(N_CORES)), trace=_trace
        )
    except Exception:
        # sporadic NRT_EXEC_UNIT_UNRECOVERABLE transients recover on retry
        res = bass_utils.run_bass_kernel_spmd(
            nc, in_maps, core_ids=list(range(N_CORES)), trace=_trace
        )
    out = np.concatenate([res.results[c]["out"] for c in range(N_CORES)], axis=0)
    if _trace:
        kernel.last_results = res
    return out
